# revision 1
# baseline (speedup 1.0000x reference)
"""Trainium2 Bass kernel for nn_DilatedSpatioTemporalGCN.

Sharding: time axis T=64 across 8 cores (8 timesteps each) for the
per-timestep adjacency + GCN; dilated temporal conv halos move via one
small AllGather per layer (with the X[-1] broadcast folded into the same
collective). Final attention computed on every core; core 7's output (the
only one with the real t=63 chain) is returned.

Self-contained: needs numpy + the concourse tree at /opt/trn_rl_repo.
"""
import sys

for _p in ('/opt/trn_rl_repo', '/root/.axon_site/_ro/trn_rl_repo'):
    if _p not in sys.path:
        sys.path.insert(0, _p)

import numpy as np

import concourse.bass as bass
import concourse.mybir as mybir
import concourse.tile as tile
from concourse.bass_utils import run_bass_kernel_spmd

# ---------------------------------------------------------------------------
# Workaround: this walrus build rejects >1 sem wait on the Tile tail drain
# (CTRL-class instruction). Split the drain's waits across single-wait NOPs.
import bass_rust
from concourse.vector_clock import ScopedClock


def _patched_drain_and_barrier(self, tick_clock, wait_clock):
    drain_inst = self.nc.sync.drain()
    wait_clock.add_sem_waits(
        drain_inst.ins, ScopedClock({None: tick_clock.global_clock})
    )
    si = drain_inst.ins.sync_info
    waits = list(si.on_wait)
    if len(waits) > 1:
        drain_inst.ins.sync_info = bass_rust.SyncInfo(
            on_wait=[waits[0]], on_update=list(si.on_update)
        )
        for w in waits[1:]:
            nop = self.nc.sync.nop(nofuse=True, hint="split_drain_wait")
            nop.ins.sync_info = bass_rust.SyncInfo(on_wait=[w], on_update=[])

    self.nc.all_engine_barrier()
    assert self.sems is not None
    popped = self.nc._tile_sem_poison_stack.pop()
    assert popped is self._sem_poison
    self.nc.clear_and_free_semaphores(list(self.sems.allocated().values()))


tile.TileContext._drain_and_barrier = _patched_drain_and_barrier


def _split_multi_waits(nc):
    """This walrus build accepts at most one sync wait per instruction;
    hoist extra waits onto same-engine NOPs inserted just before."""
    for f in nc.m.functions:
        for bb in f.blocks:
            out = []
            for ins in bb.instructions:
                si = ins.sync_info
                if (si is not None and len(si.on_wait) > 1
                        and ins.engine is not None):
                    waits = list(si.on_wait)
                    for idx, w in enumerate(waits[:-1]):
                        nop = mybir.InstNoOp(name=f"{ins.name}_w{idx}",
                                             engine=ins.engine)
                        nop.sync_info = bass_rust.SyncInfo(
                            on_wait=[w], on_update=[])
                        out.append(nop)
                    ins.sync_info = bass_rust.SyncInfo(
                        on_wait=[waits[-1]], on_update=list(si.on_update))
                out.append(ins)
            bb.instructions = out
# ---------------------------------------------------------------------------

T, N, D = 64, 1000, 64
KSZ = 3
DILS = (1, 2, 4)
L = 3
NCORES = 8
TLOC = T // NCORES            # 8 local timesteps
HALO = (3, 5)                 # halo slices shipped after layers 0, 1
NCH = 8                       # n-chunks
CH_SZ = [128] * 7 + [104]
CH_OFF = [128 * c for c in range(NCH)]
MH = [(0, 500), (500, 500)]   # m halves for contiguous SBUF operands
MHP = [(0, 500), (512, 500)]  # m halves for PSUM outputs (bank-aligned)


def _ps3(ap):
    """[P, 1024] PSUM view -> [P, 2, 500] skipping the bank-pad gap."""
    return ap.rearrange("p (h q) -> p h q", h=2)[:, :, 0:500]


def _cs3(ap):
    """Contiguous [P, 1000] view -> [P, 2, 500] matching _ps3."""
    return ap.rearrange("p (h q) -> p h q", h=2)

f32 = mybir.dt.float32
f32r = mybir.dt.float32r
f16 = mybir.dt.float16
i32 = mybir.dt.int32
AF = mybir.ActivationFunctionType
OP = mybir.AluOpType


def _prow(i):
    """Parity row-slice for local timestep i (for K-strips / conv outputs)."""
    return slice(0, 64) if i % 2 == 0 else slice(64, 128)


def _pcol(i):
    """x_sb/x_h pair-packed column block for local timestep i."""
    p = i // 2
    return slice(1000 * p, 1000 * p + 1000)


def _tcol(i):
    """Unpaired per-t column block (base-0 [64, 8000] tensors)."""
    return slice(1000 * i, 1000 * i + 1000)


def build_program(alpha: float):
    nc = bass.Bass()

    # ---- external I/O ----------------------------------------------------
    mte = nc.dram_tensor("mte", [TLOC, N, N], f32r, kind="ExternalInput")
    x_loc = nc.dram_tensor("x_loc", [128, 4000], f32, kind="ExternalInput")
    x63h_in = nc.dram_tensor("x63h_in", [64, N], f16, kind="ExternalInput")
    xprev_in = nc.dram_tensor("xprev_in", [64, N], f32, kind="ExternalInput")
    bdup = nc.dram_tensor("bdup", [128, 64], f16, kind="ExternalInput")
    gw_in = [nc.dram_tensor(f"gw{l}", [128, 64], f16, kind="ExternalInput")
             for l in range(L)]
    cw_in = [nc.dram_tensor(f"cw{l}", [64, 192], f16, kind="ExternalInput")
             for l in range(L)]
    gbias = nc.dram_tensor("gbias", [64, L], f32, kind="ExternalInput")
    cbias = nc.dram_tensor("cbias", [128, L], f32, kind="ExternalInput")
    onesh_in = nc.dram_tensor("onesh_in", [128, 512], f16, kind="ExternalInput")
    hoff0_in = nc.dram_tensor("hoff0_in", [64, 8], i32, kind="ExternalInput")
    hoff1_in = nc.dram_tensor("hoff1_in", [64, 8], i32, kind="ExternalInput")
    hmask_in = nc.dram_tensor("hmask_in", [64, 1], f32, kind="ExternalInput")
    ident_in = nc.dram_tensor("ident_in", [128, 64], f32, kind="ExternalInput")
    attp = nc.dram_tensor("attp", [128, 4], f32, kind="ExternalInput")
    out_t = nc.dram_tensor("out", [N, D], f32, kind="ExternalOutput")

    beta = [1.0 - alpha, 1.0, 1.0]
    alph = [alpha, 0.0, 0.0]

    with tile.TileContext(nc) as tc:
        # ---- persistent SBUF tiles (one long-lived pool) ----------------
        perm = tc.alloc_tile_pool(name="perm", bufs=1)

        def ptile(shape, dtype, name):
            return perm.tile(shape, dtype, name=name, tag=name)

        x_sb = ptile([128, 4000], f32, "x_sb")      # pair-packed X (fp32)
        x_h = ptile([128, 4000], f16, "x_h")        # pair-packed X (fp16)
        x63_h = ptile([128, N], f16, "x63_h")   # rows 64:128 stay zero
        xprev = ptile([64, N], f32, "xprev")
        w_bdup = ptile([128, 64], f16, "w_bdup")
        w_gw = ptile([128, 64 * L], f16, "w_gw")
        w_cw = ptile([64, 192 * L], f16, "w_cw")
        w_gb = ptile([64, L], f32, "w_gb")
        w_cb = ptile([128, L], f32, "w_cb")
        onesh = ptile([128, 512], f16, "onesh")
        hoffs = ptile([64, 16], i32, "hoffs")
        hmask = ptile([64, 1], f32, "hmask")
        ident = ptile([128, 64], f32, "ident")
        attw = ptile([128, 4], f32, "attw")

        xg_h = ptile([64, TLOC * N], f16, "xg_h")       # per-t, base 0
        halo_h = ptile([64, 5 * N], f16, "halo_h")
        xtb_h = ptile([128, TLOC * N], f16, "xtb_h")    # rows 64:128 zero
        xwones = ptile([128, TLOC * 1024], f16, "xwones")
        xw32 = ptile([128, TLOC * 512], f32r, "xw32")
        z_sb = ptile([64, TLOC * N], f16, "z_sb")       # per-t, base 0

        # p-chain (virtual timestep just before this core's range)
        x1p = ptile([64, N], f32, "x1p")
        x2p_h = ptile([64, N], f16, "x2p_h")
        xtbp_h = ptile([128, N], f16, "xtbp_h")
        xwonesp = ptile([128, 1024], f16, "xwonesp")
        zp = ptile([64, N], f16, "zp")
        xgp_h = ptile([64, N], f16, "xgp_h")

        res = [ptile([128, N], f32, f"res{l}") for l in range(L)]
        ODD = slice(64, 128)    # parity of local t = 7 (where res lives)

        # ---- DRAM tiles for collectives ---------------------------------
        with tc.tile_pool(name="dram", bufs=1, space="DRAM") as dram_pool:
            agin = [
                dram_pool.tile([(HALO[l] + 1) * 64, N], f16,
                               name=f"agin{l}")
                for l in range(2)
            ]
            agout = [
                dram_pool.tile([NCORES * (HALO[l] + 1) * 64, N], f16,
                               addr_space="Shared", name=f"agout{l}")
                for l in range(2)
            ]
            dgin = dram_pool.tile([64, 16], f16, name="dgin")
            dgout = dram_pool.tile([NCORES * 64, 16], f16,
                                   addr_space="Shared", name="dgout")

            # ---- load constants / inputs --------------------------------
            nc.sync.dma_start(x_sb[:], x_loc[:])
            nc.sync.dma_start(x63_h[0:64, :], x63h_in[:])
            # zero-pad the score contraction to k=128 so those matmuls
            # count as full-array activity (k=64 never reaches warm clock)
            nc.vector.memset(x63_h[64:128, :], 0.0)
            nc.vector.memset(xtb_h[64:128, :], 0.0)
            nc.vector.memset(xtbp_h[64:128, :], 0.0)
            nc.sync.dma_start(xprev[:], xprev_in[:])
            nc.sync.dma_start(w_bdup[:], bdup[:])
            for l in range(L):
                nc.sync.dma_start(w_gw[:, 64 * l:64 * l + 64], gw_in[l][:])
                nc.sync.dma_start(w_cw[:, 192 * l:192 * l + 192], cw_in[l][:])
            nc.sync.dma_start(w_gb[:], gbias[:])
            nc.sync.dma_start(w_cb[:], cbias[:])
            nc.sync.dma_start(onesh[:], onesh_in[:])
            nc.sync.dma_start(hoffs[:, 0:8], hoff0_in[:])
            nc.sync.dma_start(hoffs[:, 8:16], hoff1_in[:])
            nc.sync.dma_start(hmask[:], hmask_in[:])
            nc.sync.dma_start(ident[:], ident_in[:])
            nc.sync.dma_start(attw[:], attp[:])

            # warm up the collective runtime early (overlaps layer-0 work)
            with tc.tile_pool(name="warm", bufs=1) as wpool:
                wtile = wpool.tile([64, 16], f16, name="wtile")
                nc.vector.memset(wtile[:], 0.0)
                nc.sync.dma_start(dgin[:], wtile[:])
                nc.gpsimd.collective_compute(
                    "AllGather", OP.bypass,
                    replica_groups=[list(range(NCORES))],
                    ins=[dgin[:]], outs=[dgout[:]],
                )

            # ---- helpers ------------------------------------------------
            def gcn_t(li, tag, tidx, xtb_ap, xwo, xw32_off, z_ap, xg_dst,
                      psSC, psGE, psGM, psVB, eP, wkP, mteS, sc_bufs):
                """Full GCN for one timestep (all tiles at base 0):
                scores -> E -> message/denominator -> combine -> xg."""
                bl = beta[li]
                al = alph[li]
                ge = psGE.tile([128, 1024], f32, name=f"ge_{tag}", tag="ge")
                gm = None
                if li == 0:
                    gm = psGM.tile([64, 1024], f32, name=f"gm_{tag}",
                                   tag="gm")
                for c in range(NCH):
                    sz = CH_SZ[c]
                    co = CH_OFF[c]
                    sc = psSC.tile([128, 1024], f32, name=f"sc_{tag}_{c}",
                                   tag="sc", bufs=sc_bufs)
                    for (po, _), (mo, ms) in zip(MHP, MH):
                        nc.tensor.matmul(
                            sc[0:sz, po:po + ms],
                            xtb_ap[:, co:co + sz],
                            x63_h[:, mo:mo + ms],
                            start=True, stop=True,
                        )
                    e_c = eP.tile([128, N], f16, name=f"e_{tag}_{c}",
                                  tag="e", bufs=4)
                    nc.scalar.activation(_cs3(e_c[0:sz, :]),
                                         _ps3(sc[0:sz, :]), AF.Exp)
                    nc.vector.tensor_scalar_max(e_c[0:sz, :], e_c[0:sz, :],
                                                1.0)
                    for (po, _), (mo, ms) in zip(MHP, MH):
                        nc.tensor.matmul(
                            ge[:, po:po + ms],
                            xwo[0:sz, 128 * c:128 * c + 128],
                            e_c[0:sz, mo:mo + ms],
                            start=(c == 0), stop=(c == NCH - 1),
                        )
                    if li == 0:
                        mt = mteS.tile([128, N], f32r, name=f"mt_{tag}_{c}",
                                       tag="mt", bufs=2)
                        nc.sync.dma_start(mt[0:sz, :],
                                          mte[tidx, co:co + sz, :])
                        for (po, _), (mo, ms) in zip(MHP, MH):
                            nc.tensor.matmul(
                                gm[:, po:po + ms],
                                xw32[0:sz,
                                     xw32_off + 64 * c:
                                     xw32_off + 64 * c + 64],
                                mt[0:sz, mo:mo + ms],
                                start=(c == 0), stop=(c == NCH - 1),
                            )
                # v = 2 - D/N on rows 64:128 (fp16), then a K=1 matmul
                # broadcasts row 64 down to partitions 0:64 in PSUM.
                vtmp = wkP.tile([128, N], f16, name=f"vt_{tag}", tag="wkv",
                                bufs=2)
                nc.scalar.activation(_cs3(vtmp[64:128, :]),
                                     _ps3(ge[64:128, :]), AF.Copy,
                                     bias=2.0, scale=-1.0 / N)
                vps = psVB.tile([64, 1024], f32, name=f"vp_{tag}", tag="vb")
                for (po, _), (mo, ms) in zip(MHP, MH):
                    nc.tensor.matmul(
                        vps[:, po:po + ms],
                        onesh[64:65, 0:64],
                        vtmp[64:65, mo:mo + ms],
                        start=True, stop=True,
                        tile_position=(64, 0),
                    )
                u_t = wkP.tile([64, N], f32, name=f"u_{tag}", tag="wku",
                               bufs=2)
                nc.vector.tensor_scalar(_cs3(u_t[:]), _ps3(ge[0:64, :]),
                                        bl / (2.0 * N), None, OP.mult)
                w_t = wkP.tile([64, N], f32, name=f"w_{tag}", tag="wkw",
                               bufs=2)
                nc.vector.tensor_tensor(_cs3(w_t[:]), _cs3(u_t[:]),
                                        _ps3(vps[:]), OP.mult)
                if li == 0:
                    g_t = wkP.tile([64, N], f32, name=f"g_{tag}", tag="wkg",
                                   bufs=2)
                    nc.vector.tensor_scalar(_cs3(g_t[:]), _ps3(gm[:]),
                                            al / 2.0, None, OP.mult)
                    nc.vector.tensor_add(w_t[:], w_t[:], g_t[:])
                q_t = wkP.tile([64, N], f32, name=f"q_{tag}", tag="wku",
                               bufs=2)
                nc.vector.tensor_add(q_t[:], w_t[:], z_ap)
                nc.vector.tensor_scalar_max(xg_dst, q_t[:], 0.0)

            def conv_taps(li, i, psY, use_virtual=False):
                """Accumulate the 3 causal taps for local timestep i into a
                fresh [64, N] psum region at i's parity base; negative t'
                reads halo slices (or xgp for L2's virtual tap)."""
                dil = DILS[li]
                par = i % 2
                y_ps = psY.tile([128, 1024], f32, name=f"y_{li}_{i}",
                                tag="y", bufs=2)
                orows = _prow(i)
                for k in range(KSZ):
                    tp = i - (KSZ - 1 - k) * dil
                    if tp >= 0:
                        rhs = xg_h[:, _tcol(tp)]
                    elif use_virtual and tp == -1:
                        rhs = xgp_h[:]
                    else:
                        slot = HALO[li] + tp
                        assert 0 <= slot < HALO[li], (li, i, tp)
                        rhs = halo_h[:, N * slot:N * slot + N]
                    for (po, _), (mo, ms) in zip(MHP, MH):
                        nc.tensor.matmul(
                            y_ps[orows, po:po + ms],
                            w_cw[:, 192 * li + 64 * k:192 * li + 64 * k + 64],
                            rhs[:, mo:mo + ms],
                            start=(k == 0), stop=(k == KSZ - 1),
                            tile_position=(0, 64 * par),
                        )
                return y_ps, orows

            # ========================= layers =============================
            for li in range(L):
                H = HALO[li] if li < 2 else 0
                dil = DILS[li]

                # ---- phase 0/1: casts + XtB / XW / XWT / z ---------------
                with tc.tile_pool(name=f"psA{li}", bufs=1,
                                  space="PSUM") as psA:
                    for i in range(TLOC):
                        par = i % 2
                        rs = _prow(i)
                        pcol = _pcol(i)
                        tcol = _tcol(i)
                        pco = 1000 * (i // 2)
                        if par == 0:
                            nc.vector.tensor_copy(x_h[:, pcol], x_sb[:, pcol])
                        big = psA.tile([64, 1024], f32,
                                       name=f"xtbps{li}{i}",
                                       tag="big", bufs=2)
                        for (po, _), (mo, ms) in zip(MHP, MH):
                            nc.tensor.matmul(
                                big[:, po:po + ms],
                                w_bdup[rs, :],
                                x_h[rs, pco + mo:pco + mo + ms],
                                start=True, stop=True,
                                tile_position=(64 * par, 0),
                            )
                        nc.vector.tensor_copy(_cs3(xtb_h[0:64, tcol]),
                                              _ps3(big[:]))

                        bigT = psA.tile([64, 1024], f32,
                                        name=f"xwtps{li}{i}",
                                        tag="big", bufs=2)
                        for (po, _), (mo, ms) in zip(MHP, MH):
                            nc.tensor.matmul(
                                bigT[:, po:po + ms],
                                w_gw[rs, 64 * li:64 * li + 64],
                                x_h[rs, pco + mo:pco + mo + ms],
                                start=True, stop=True,
                                tile_position=(64 * par, 0),
                            )
                        nc.vector.tensor_scalar(
                            _cs3(z_sb[:, tcol]), _ps3(bigT[:]),
                            0.5, w_gb[:, li:li + 1],
                            OP.mult, OP.add,
                        )

                        xwp = psA.tile([128, 512], f32, name=f"xwps{li}{i}",
                                       tag="xw", bufs=2)
                        for c in range(NCH):
                            sz = CH_SZ[c]
                            co = CH_OFF[c]
                            nc.tensor.matmul(
                                xwp[0:sz, 64 * c:64 * c + 64],
                                x_h[rs, pco + co:pco + co + sz],
                                w_gw[rs, 64 * li:64 * li + 64],
                                start=True, stop=True,
                                tile_position=(64 * par, 0),
                            )
                        nc.vector.tensor_copy(
                            xwones[:, 1024 * i:1024 * i + 1024]
                            .rearrange("p (c x) -> p c x", c=NCH)
                            [:, :, 0:64],
                            xwp[:].rearrange("p (c x) -> p c x", c=NCH),
                        )
                        nc.vector.tensor_copy(
                            xwones[:, 1024 * i:1024 * i + 1024]
                            .rearrange("p (c x) -> p c x", c=NCH)
                            [:, :, 64:128],
                            onesh[:].rearrange("p (c x) -> p c x", c=NCH),
                        )
                        if li == 0:
                            nc.vector.tensor_copy(
                                xw32[:, 512 * i:512 * i + 512],
                                xwp[:],
                            )

                    # L2 p-chain phase 0/1 from x2p_h (base 0 everywhere)
                    if li == 2:
                        bigp = psA.tile([64, 1024], f32, name="xtbpp",
                                        tag="big", bufs=2)
                        for (po, _), (mo, ms) in zip(MHP, MH):
                            nc.tensor.matmul(
                                bigp[:, po:po + ms], w_bdup[0:64, :],
                                x2p_h[:, mo:mo + ms], start=True, stop=True)
                        nc.vector.tensor_copy(_cs3(xtbp_h[0:64, :]),
                                              _ps3(bigp[:]))
                        bigp2 = psA.tile([64, 1024], f32, name="xwtpp",
                                         tag="big", bufs=2)
                        for (po, _), (mo, ms) in zip(MHP, MH):
                            nc.tensor.matmul(
                                bigp2[:, po:po + ms],
                                w_gw[0:64, 64 * li:64 * li + 64],
                                x2p_h[:, mo:mo + ms], start=True, stop=True)
                        nc.vector.tensor_scalar(
                            _cs3(zp[:]), _ps3(bigp2[:]), 0.5,
                            w_gb[:, li:li + 1],
                            OP.mult, OP.add)
                        xwpp = psA.tile([128, 512], f32, name="xwpsp",
                                        tag="xw", bufs=2)
                        for c in range(NCH):
                            sz = CH_SZ[c]
                            co = CH_OFF[c]
                            nc.tensor.matmul(
                                xwpp[0:sz, 64 * c:64 * c + 64],
                                x2p_h[:, co:co + sz],
                                w_gw[0:64, 64 * li:64 * li + 64],
                                start=True, stop=True)
                        nc.vector.tensor_copy(
                            xwonesp[:].rearrange("p (c x) -> p c x", c=NCH)
                            [:, :, 0:64],
                            xwpp[:].rearrange("p (c x) -> p c x", c=NCH))
                        nc.vector.tensor_copy(
                            xwonesp[:].rearrange("p (c x) -> p c x", c=NCH)
                            [:, :, 64:128],
                            onesh[:].rearrange("p (c x) -> p c x", c=NCH))

                # ---- phase 2: per-t GCN ----------------------------------
                sc_bufs = 1 if li == 0 else 2
                with (
                    tc.tile_pool(name=f"psSC{li}", bufs=1,
                                 space="PSUM") as psSC,
                    tc.tile_pool(name=f"psGE{li}", bufs=1,
                                 space="PSUM") as psGE,
                    tc.tile_pool(name=f"psGM{li}", bufs=1,
                                 space="PSUM") as psGM,
                    tc.tile_pool(name=f"psVB{li}", bufs=1,
                                 space="PSUM") as psVB,
                    tc.tile_pool(name=f"eP{li}", bufs=1) as eP,
                    tc.tile_pool(name=f"wkP{li}", bufs=1) as wkP,
                    tc.tile_pool(name=f"mteS{li}", bufs=1) as mteS,
                ):
                    for i in range(TLOC):
                        gcn_t(li, f"l{li}t{i}", i,
                              xtb_h[:, _tcol(i)],
                              xwones[:, 1024 * i:1024 * i + 1024],
                              512 * i,
                              z_sb[:, _tcol(i)],
                              xg_h[:, _tcol(i)],
                              psSC, psGE, psGM, psVB, eP, wkP, mteS, sc_bufs)
                    if li == 2:
                        gcn_t(li, "l2tp", 0,
                              xtbp_h[:],
                              xwonesp[:],
                              0,
                              zp[:],
                              xgp_h[:],
                              psSC, psGE, psGM, psVB, eP, wkP, mteS, sc_bufs)

                # ---- phase 3: conv + AG + X update -----------------------
                with (
                    tc.tile_pool(name=f"psY{li}", bufs=1,
                                 space="PSUM") as psY,
                    tc.tile_pool(name=f"yP{li}", bufs=1) as yP,
                ):
                    if li < 2:
                        # (a) no-halo timesteps first (incl. last local t)
                        for i in range(2 * dil, TLOC):
                            y_ps, orows = conv_taps(li, i, psY)
                            y_t = yP.tile([128, N], f32, name=f"yt{li}{i}",
                                          tag="yt", bufs=3)
                            nc.vector.tensor_scalar(
                                _cs3(y_t[orows, :]), _ps3(y_ps[orows, :]),
                                w_cb[orows, li:li + 1], 0.0,
                                OP.add, OP.max)
                            if i == TLOC - 1:
                                nc.vector.tensor_copy(res[li][ODD, :],
                                                      y_t[ODD, :])
                                xc = yP.tile([128, N], f16, name=f"xc{li}",
                                             tag="xc", bufs=1)
                                nc.vector.tensor_tensor(
                                    xc[ODD, :], y_t[ODD, :],
                                    x_sb[ODD, _pcol(i)], OP.add)
                                nc.sync.dma_start(
                                    agin[li][H * 64:(H + 1) * 64, :],
                                    xc[ODD, :])
                            nc.vector.tensor_tensor(
                                x_sb[orows, _pcol(i)], y_t[orows, :],
                                x_sb[orows, _pcol(i)], OP.add)
                        # (b) halo payload slices: xg[TLOC-H+k]
                        for k in range(H):
                            i = TLOC - H + k
                            nc.sync.dma_start(
                                agin[li][k * 64:(k + 1) * 64, :],
                                xg_h[:, _tcol(i)])
                        # (c) AllGather
                        nc.gpsimd.collective_compute(
                            "AllGather", OP.bypass,
                            replica_groups=[list(range(NCORES))],
                            ins=[agin[li][:]], outs=[agout[li][:]],
                        )
                        # (d) halo (left neighbor) + new X63 (from core 7)
                        for k in range(H):
                            nc.gpsimd.indirect_dma_start(
                                out=halo_h[:, N * k:N * k + N],
                                out_offset=None,
                                in_=agout[li][:],
                                in_offset=bass.IndirectOffsetOnAxis(
                                    ap=hoffs[:, 8 * li + k:8 * li + k + 1],
                                    axis=0),
                            )
                        nc.vector.tensor_scalar_mul(
                            halo_h[:, 0:N * H], halo_h[:, 0:N * H],
                            hmask[:])
                        x63row = (7 * (H + 1) + H) * 64
                        nc.sync.dma_start(
                            x63_h[0:64, :],
                            agout[li][x63row:x63row + 64, :])
                        # (e) halo timesteps' conv + X update
                        for i in range(0, 2 * dil):
                            y_ps, orows = conv_taps(li, i, psY)
                            y_t = yP.tile([128, N], f32, name=f"yt{li}{i}",
                                          tag="yt", bufs=3)
                            nc.vector.tensor_scalar(
                                _cs3(y_t[orows, :]), _ps3(y_ps[orows, :]),
                                w_cb[orows, li:li + 1], 0.0,
                                OP.add, OP.max)
                            nc.vector.tensor_tensor(
                                x_sb[orows, _pcol(i)], y_t[orows, :],
                                x_sb[orows, _pcol(i)], OP.add)
                        # (f) p-chain update (all base 0)
                        if li == 0:
                            yp_ps = psY.tile([128, 1024], f32, name="y0p",
                                             tag="y", bufs=2)
                            for k in range(KSZ):
                                slot = k          # tp = -3 + k
                                for (po, _), (mo, ms) in zip(MHP, MH):
                                    nc.tensor.matmul(
                                        yp_ps[0:64, po:po + ms],
                                        w_cw[:, 64 * k:64 * k + 64],
                                        halo_h[:, N * slot + mo:
                                               N * slot + mo + ms],
                                        start=(k == 0), stop=(k == KSZ - 1))
                            ypt = yP.tile([128, N], f32, name="y0pt",
                                          tag="yt", bufs=3)
                            nc.vector.tensor_scalar(
                                _cs3(ypt[0:64, :]), _ps3(yp_ps[0:64, :]),
                                w_cb[0:64, li:li + 1],
                                0.0, OP.add, OP.max)
                            nc.vector.tensor_add(x1p[:], ypt[0:64, :],
                                                 xprev[:])
                        elif li == 1:
                            yp_ps = psY.tile([128, 1024], f32, name="y1p",
                                             tag="y", bufs=2)
                            for k in range(KSZ):
                                slot = 2 * k      # tp = -5 + 2k
                                for (po, _), (mo, ms) in zip(MHP, MH):
                                    nc.tensor.matmul(
                                        yp_ps[0:64, po:po + ms],
                                        w_cw[:, 192 + 64 * k:
                                             192 + 64 * k + 64],
                                        halo_h[:, N * slot + mo:
                                               N * slot + mo + ms],
                                        start=(k == 0), stop=(k == KSZ - 1))
                            ypt = yP.tile([128, N], f32, name="y1pt",
                                          tag="yt", bufs=3)
                            nc.vector.tensor_scalar(
                                _cs3(ypt[0:64, :]), _ps3(yp_ps[0:64, :]),
                                w_cb[0:64, li:li + 1],
                                0.0, OP.add, OP.max)
                            x2p = yP.tile([128, N], f32, name="x2p",
                                          tag="yt", bufs=3)
                            nc.vector.tensor_add(x2p[0:64, :], ypt[0:64, :],
                                                 x1p[:])
                            nc.vector.tensor_copy(x2p_h[:], x2p[0:64, :])
                    else:
                        # L2: res_2 = conv at local last t (t'=-1 -> xgp)
                        y_ps, orows = conv_taps(li, TLOC - 1, psY,
                                                use_virtual=True)
                        nc.vector.tensor_scalar(
                            _cs3(res[2][orows, :]), _ps3(y_ps[orows, :]),
                            w_cb[orows, li:li + 1], 0.0,
                            OP.add, OP.max)

            # ===================== attention (all cores) ==================
            with (
                tc.tile_pool(name="psAT", bufs=1, space="PSUM") as psAT,
                tc.tile_pool(name="atP", bufs=1) as atP,
            ):
                S_t = []
                for m in range(L):
                    sm = atP.tile([128, N], f32, name=f"S{m}", tag="S",
                                  bufs=3)
                    nc.scalar.activation(
                        sm[ODD, :], res[m][ODD, :], AF.Tanh,
                        bias=attw[ODD, 1:2], scale=attw[ODD, 0:1])
                    S_t.append(sm)
                for c in range(NCH):
                    sz = CH_SZ[c]
                    co = CH_OFF[c]
                    s_ps = psAT.tile([128, 4], f32, name=f"sps{c}",
                                     tag="sps", bufs=2)
                    for m in range(L):
                        nc.tensor.matmul(
                            s_ps[0:sz, m:m + 1], S_t[m][ODD, co:co + sz],
                            attw[ODD, 2:3], start=True, stop=True,
                            tile_position=(64, 0))
                    e_s = atP.tile([128, 4], f32, name=f"es{c}", tag="es",
                                   bufs=2)
                    nc.scalar.activation(e_s[0:sz, 0:3], s_ps[0:sz, 0:3],
                                         AF.Exp)
                    den = atP.tile([128, 1], f32, name=f"den{c}", tag="den",
                                   bufs=2)
                    nc.vector.tensor_reduce(
                        den[0:sz, :], e_s[0:sz, 0:3], mybir.AxisListType.X,
                        OP.add)
                    rden = atP.tile([128, 1], f32, name=f"rden{c}",
                                    tag="rden", bufs=2)
                    nc.vector.reciprocal(rden[0:sz, :], den[0:sz, :])
                    aw = atP.tile([128, 4], f32, name=f"aw{c}", tag="aw",
                                  bufs=2)
                    nc.vector.tensor_scalar_mul(
                        aw[0:sz, 0:3], e_s[0:sz, 0:3], rden[0:sz, :])
                    acc = atP.tile([128, 64], f32, name=f"acc{c}",
                                   tag="acc", bufs=2)
                    for m in range(L):
                        rt_ps = psAT.tile([128, 64], f32, name=f"rt{c}{m}",
                                          tag="rt", bufs=2)
                        nc.tensor.transpose(
                            rt_ps[0:sz, :], res[m][ODD, co:co + sz],
                            ident[ODD, :], tile_position=(64, 0))
                        if m == 0:
                            nc.vector.tensor_scalar_mul(
                                acc[0:sz, :], rt_ps[0:sz, :],
                                aw[0:sz, m:m + 1])
                        else:
                            tmp = atP.tile([128, 64], f32, name=f"tmp{c}{m}",
                                           tag="tmp", bufs=2)
                            nc.vector.tensor_scalar_mul(
                                tmp[0:sz, :], rt_ps[0:sz, :],
                                aw[0:sz, m:m + 1])
                            nc.vector.tensor_add(
                                acc[0:sz, :], acc[0:sz, :], tmp[0:sz, :])
                    nc.sync.dma_start(out_t[co:co + sz, :], acc[0:sz, :])

        perm.release()

    _split_multi_waits(nc)
    return nc


# ---------------------------------------------------------------------------
def _prep_inputs(inputs):
    """Build the 8 per-core input maps from the full problem inputs."""
    X = np.ascontiguousarray(np.asarray(inputs["node_embeddings"], np.float32))
    B = np.asarray(inputs["B_weight"], np.float32)
    MTE = np.asarray(inputs["static_MTE"], np.float32)
    gcn_W = np.asarray(inputs["gcn_W"], np.float32)
    gcn_b = np.asarray(inputs["gcn_b"], np.float32)
    conv_W = np.asarray(inputs["conv_W"], np.float32)
    conv_b = np.asarray(inputs["conv_b"], np.float32)
    W_a = np.asarray(inputs["W_a"], np.float32)
    v = np.asarray(inputs["v"], np.float32)
    b_a = np.asarray(inputs["b_a"], np.float32)

    def dup(a):  # [64, k] -> [128, k]
        return np.ascontiguousarray(np.concatenate([a, a], axis=0))

    bdup = dup(B.astype(np.float16))
    gws = [dup(gcn_W[l].astype(np.float16)) for l in range(L)]
    cws = []
    for l in range(L):
        blocks = [np.ascontiguousarray(conv_W[l, :, :, k].T).astype(np.float16)
                  for k in range(KSZ)]
        cws.append(np.ascontiguousarray(np.concatenate(blocks, axis=1)))
    gbias = np.ascontiguousarray(gcn_b.T).astype(np.float32)        # [64, 3]
    cbias = dup(np.ascontiguousarray(conv_b.T).astype(np.float32))  # [128, 3]
    onesh = np.ones((128, 512), np.float16)
    ident = dup(np.eye(64, dtype=np.float32))                       # [128, 64]
    attp = np.zeros((64, 4), np.float32)
    attp[:, 0] = np.diagonal(W_a)
    attp[:, 1] = b_a
    attp[:, 2] = v[:, 0]
    attp = dup(attp)                                                # [128, 4]

    x63h = np.ascontiguousarray(X[T - 1].T).astype(np.float16)      # [64, N]

    in_maps = []
    for j in range(NCORES):
        Xl = X[TLOC * j:TLOC * (j + 1)]              # [8, N, D]
        xl = np.zeros((128, 4000), np.float32)
        for i in range(TLOC):
            rs = slice(0, 64) if i % 2 == 0 else slice(64, 128)
            xl[rs, 1000 * (i // 2):1000 * (i // 2) + 1000] = Xl[i].T
        xprev = (np.ascontiguousarray(X[TLOC * j - 1].T) if j > 0
                 else np.zeros((D, N), np.float32)).astype(np.float32)
        hoffs = []
        for H in HALO:
            src = (j - 1) % NCORES
            off = np.zeros((64, 8), np.int32)
            for k in range(H):
                off[:, k] = (src * (H + 1) + k) * 64 + np.arange(64)
            hoffs.append(off)
        hmask = np.full((64, 1), 0.0 if j == 0 else 1.0, np.float32)
        in_maps.append({
            "mte": np.ascontiguousarray(MTE[TLOC * j:TLOC * (j + 1)]),
            "x_loc": xl,
            "x63h_in": x63h,
            "xprev_in": np.ascontiguousarray(xprev),
            "bdup": bdup,
            **{f"gw{l}": gws[l] for l in range(L)},
            **{f"cw{l}": cws[l] for l in range(L)},
            "gbias": gbias,
            "cbias": cbias,
            "onesh_in": onesh,
            "hoff0_in": hoffs[0],
            "hoff1_in": hoffs[1],
            "hmask_in": hmask,
            "ident_in": ident,
            "attp": attp,
        })
    return in_maps


_prog_cache = {}


def run_kernel(inputs, trace=False):
    alpha = float(np.asarray(inputs["alpha"]))
    if alpha not in _prog_cache:
        _prog_cache[alpha] = build_program(alpha)
    nc = _prog_cache[alpha]
    in_maps = _prep_inputs(inputs)
    res = run_bass_kernel_spmd(nc, in_maps, list(range(NCORES)), trace=trace)
    out = np.asarray(res.results[7]["out"], np.float32)
    return out, res


def kernel(**inputs) -> np.ndarray:
    out, _ = run_kernel(inputs, trace=False)
    return out

